# revision 1
# baseline (speedup 1.0000x reference)
"""Trainium2 Bass kernel for nn_Decoder: fused single-step LSTM decoder.

Reference computation (per token t of batch b, state never advances):
    gates = x[b,t] @ W_ih.T + (h0[b] @ W_hh.T + b_ih + b_hh)     # [2048]
    i, f, g, o = sigmoid/sigmoid/tanh/sigmoid of gate quarters
    c = f * c0[b] + i * g
    h = o * tanh(c)
    out[b,t] = h @ fc_w.T + fc_b                                 # [513]

Sharding: data-parallel, batch 64 -> 8 batches per core on 8 NeuronCores.
Per-core layout strategy ("feature-major"):
  - x is cast to bf16 on the host and zero-padded to 640 columns; the
    kernel never loads it natively. Each 512-token tile is brought in as
    5 DMA xbar-transposes (DRAM [512 tok, 128 d] -> SBUF [128 d, 512]),
    so the transpose costs zero PE/DVE time. The 5th window [512:640)
    holds feature 512 at row 0 (rest zero padding).
  - gates are computed transposed in bf16: gatesT[g-chunk, tok] with
    host-cast bf16 W_ihT as the stationary operand; the K=513
    contraction is 4 full K=128 chunks + one K=1 matmul for feature
    512. The per-batch bias const (h0 @ W_hh.T + b_ih + b_hh,
    precomputed fp32 on host) enters for free as the ScalarE activation
    per-partition bias AP.
  - LSTM cell math runs feature-major in fp32 so c0 is a per-partition
    scalar; h (cast to bf16) lands directly in the [h, tok] layout the
    fc matmul needs as lhsT.
  - fc runs in bf16 on h: out[tok, 513] token-major in two N-halves
    (258+258 into one 2-bank PSUM tile, cols 513..515 zero-pad), fc_b
    folded in via a K=1 ones-row matmul. One ScalarE copy PSUM->SBUF
    per subtile, DMA out per 128-token subtile.
  - All DMAs (including the xbar transposes) issue on the single SP
    queue: mixing transpose-mode and copy-mode DMAs across queues hits
    a TRN2 xbar hardware bug (silent corruption or device hang).
"""

from contextlib import ExitStack

import ml_dtypes
import numpy as np

import concourse.bass as bass
import concourse.tile as tile
from concourse import bacc, mybir
from concourse.bass_utils import run_bass_kernel_spmd

FP32 = mybir.dt.float32
FP32R = mybir.dt.float32r
BF16 = mybir.dt.bfloat16
AFT = mybir.ActivationFunctionType

N_CORES = 8
B, T, D = 64, 1024, 513
H = 512
G4 = 4 * H  # 2048
B_LOC = B // N_CORES  # 8 batches per core
TOK = B_LOC * T  # 8192 tokens per core
TT = 512  # tokens per tile (stays within one batch: T % TT == 0)
NT = TOK // TT  # 16 tiles
NM = TT // 128  # 4 token-subtiles of 128
DX = 640  # host-padded x width: 5 transpose windows of 128 (cols 513+ zero)
DPAD = 516  # fc output padded width (cols 513..515 zero garbage)
NSPLIT = [258, 258]  # fc N split halves (each fits one PSUM bank)


def build_nc(reps=1):
    nc = bacc.Bacc("TRN2", target_bir_lowering=False, debug=False, num_devices=N_CORES)
    x = nc.dram_tensor("x", [TOK, DX], BF16, kind="ExternalInput").ap()
    wih_t = nc.dram_tensor("wih_t", [H, G4], BF16, kind="ExternalInput").ap()
    wih_row = nc.dram_tensor("wih_row", [1, G4], BF16, kind="ExternalInput").ap()
    fc_rhs = nc.dram_tensor("fc_rhs", [H, DPAD], BF16, kind="ExternalInput").ap()
    fc_row = nc.dram_tensor("fc_row", [1, DPAD], BF16, kind="ExternalInput").ap()
    bct = nc.dram_tensor("bconst", [128, 16 * B_LOC], FP32, kind="ExternalInput").ap()
    c0t = nc.dram_tensor("c0t", [128, 4 * B_LOC], FP32, kind="ExternalInput").ap()
    out = nc.dram_tensor("out", [TOK, D], FP32, kind="ExternalOutput").ap()

    with tile.TileContext(nc) as tc, ExitStack() as ctx:
        const = ctx.enter_context(tc.tile_pool(name="const", bufs=1))
        xtp = ctx.enter_context(tc.tile_pool(name="xt", bufs=15))
        sigp = ctx.enter_context(tc.tile_pool(name="sig", bufs=10))
        tmpp = ctx.enter_context(tc.tile_pool(name="tmp", bufs=2))
        hp = ctx.enter_context(tc.tile_pool(name="h", bufs=8))
        outp = ctx.enter_context(tc.tile_pool(name="osb", bufs=4))
        # PSUM: 8 banks: gates 4 + fc 2x2-bank
        gpp = ctx.enter_context(tc.tile_pool(name="gp", bufs=4, space="PSUM"))
        fmp = ctx.enter_context(tc.tile_pool(name="fm", bufs=2, space="PSUM"))

        def load_weights():
            wsb = []
            for k in range(4):
                w = const.tile([128, G4], BF16, tag=f"w{k}", name=f"w{k}")
                nc.sync.dma_start(w[:], wih_t[k * 128 : (k + 1) * 128, :])
                wsb.append(w)
            w4 = const.tile([1, G4], BF16, tag="w4")
            nc.sync.dma_start(w4[:], wih_row)
            wsb.append(w4)
            return wsb

        def load_fc_chunk(k):
            w = const.tile([128, DPAD], BF16, tag=f"f{k}", name=f"f{k}")
            nc.sync.dma_start(w[:], fc_rhs[k * 128 : (k + 1) * 128, :])
            return w

        gate_funcs = [AFT.Sigmoid, AFT.Sigmoid, AFT.Tanh, AFT.Sigmoid]

        def emit_transposes(tt):
            """Bring in tile tt as 5 transposed bf16 chunks via DMA xbar."""
            ts = tt * TT
            xt = []
            for k in range(5):
                t = xtp.tile([128, TT], BF16, tag="xt")
                nc.sync.dma_start_transpose(
                    t[:], x[ts : ts + TT, k * 128 : (k + 1) * 128]
                )
                xt.append(t)
            return xt

        # ---- startup, DMAs ordered by first use on the single queue:
        # xt0 (first transposes) -> W_ih (first gates) -> bct/c0 (first
        # activations) -> fc weights -> xt1 ----
        xt_queue = [emit_transposes(0)]
        wsb = load_weights()

        bct_sb = const.tile([128, 16 * B_LOC], FP32, tag="bct")
        c0_sb = const.tile([128, 4 * B_LOC], FP32, tag="c0")
        nc.sync.dma_start(bct_sb[:], bct)
        nc.sync.dma_start(c0_sb[:], c0t)

        fcsb = [load_fc_chunk(k) for k in range(4)]
        f4 = const.tile([1, DPAD], BF16, tag="f4")
        nc.sync.dma_start(f4[:], fc_row)
        fcsb.append(f4)

        ones = const.tile([1, 128], BF16, tag="ones")
        nc.vector.memset(ones[:], 1.0)

        xt_queue.append(emit_transposes(1))

        # ---- main loop over 16 token tiles of 512 ----
        # (optionally repeated `reps` times inside one NEFF for timing)
        rep_ctx = tc.For_i(0, reps, 1) if reps > 1 else None
        if rep_ctx is not None:
            rep_ctx.__enter__()
        for tt in range(NT):
            b = tt // (T // TT)
            ts = tt * TT
            xt = xt_queue.pop(0)

            # prefetch + transpose two tiles ahead while this tile computes
            # (reps>1 wraps around so each For_i iteration is steady-state)
            if reps > 1:
                xt_queue.append(emit_transposes((tt + 2) % NT))
            elif tt + 2 < NT:
                xt_queue.append(emit_transposes(tt + 2))

            # gates + LSTM cell, per h-chunk k
            hn = []
            for k in range(4):
                gs = []
                for gi in range(4):
                    c = gi * 4 + k  # g-chunk index in [0, 16)
                    ps = gpp.tile([128, TT], FP32, tag="gp")
                    for kd in range(4):
                        nc.tensor.matmul(
                            ps[:],
                            wsb[kd][:, c * 128 : (c + 1) * 128],
                            xt[kd][:],
                            start=(kd == 0),
                            stop=False,
                        )
                    nc.tensor.matmul(
                        ps[:],
                        wsb[4][:, c * 128 : (c + 1) * 128],
                        xt[4][0:1, :],
                        start=False,
                        stop=True,
                    )
                    s = sigp.tile([128, TT], FP32, tag="sig")
                    nc.scalar.activation(
                        s[:],
                        ps[:],
                        gate_funcs[gi],
                        bias=bct_sb[:, c * B_LOC + b : c * B_LOC + b + 1],
                    )
                    gs.append(s)
                i_s, f_s, g_s, o_s = gs
                t1 = tmpp.tile([128, TT], FP32, tag="t1")
                nc.vector.tensor_mul(t1[:], i_s[:], g_s[:])
                t2 = tmpp.tile([128, TT], FP32, tag="t2")
                nc.vector.tensor_scalar_mul(
                    t2[:], f_s[:], c0_sb[:, k * B_LOC + b : k * B_LOC + b + 1]
                )
                cc = tmpp.tile([128, TT], FP32, tag="cc")
                nc.vector.tensor_add(cc[:], t1[:], t2[:])
                th = tmpp.tile([128, TT], FP32, tag="th")
                nc.scalar.activation(th[:], cc[:], AFT.Tanh)
                h = hp.tile([128, TT], BF16, tag="h")
                nc.vector.tensor_mul(h[:], o_s[:], th[:])
                hn.append(h)

            # fc: out[tok, 513] per 128-token subtile, N split 256 + 258
            osb = outp.tile([128, NM, DPAD], FP32, tag="osb")
            for m in range(NM):
                msl = slice(m * 128, (m + 1) * 128)
                lhs5 = [
                    hn[0][:, msl],
                    hn[1][:, msl],
                    hn[2][:, msl],
                    hn[3][:, msl],
                    ones[:],
                ]
                pf = fmp.tile([128, 1024], FP32, tag="fm")
                for kd in range(5):
                    nc.tensor.matmul(
                        pf[:, 0 : NSPLIT[0]],
                        lhs5[kd],
                        fcsb[kd][:, 0 : NSPLIT[0]],
                        start=(kd == 0),
                        stop=(kd == 4),
                    )
                for kd in range(5):
                    nc.tensor.matmul(
                        pf[:, 512 : 512 + NSPLIT[1]],
                        lhs5[kd],
                        fcsb[kd][:, NSPLIT[0] : DPAD],
                        start=(kd == 0),
                        stop=(kd == 4),
                    )
                nc.scalar.copy(
                    osb[:, m, 0:DPAD].rearrange("p (a b) -> p a b", a=2),
                    pf[:].rearrange("p (a b) -> p a b", a=2)[:, :, 0 : NSPLIT[0]],
                )
                nc.sync.dma_start(
                    out[ts + m * 128 : ts + (m + 1) * 128, :], osb[:, m, 0:D]
                )
        if rep_ctx is not None:
            rep_ctx.__exit__(None, None, None)

    nc.compile()
    return nc


_NC_CACHE = []


def get_nc():
    if not _NC_CACHE:
        _NC_CACHE.append(build_nc())
    return _NC_CACHE[0]


def make_in_maps(decoder_inputs, h0, c0, W_ih, W_hh, b_ih, b_hh, fc_w, fc_b):
    di = np.asarray(decoder_inputs, dtype=np.float32)
    h0 = np.asarray(h0, dtype=np.float32)[0]  # [64, 512]
    c0 = np.asarray(c0, dtype=np.float32)[0]
    W_ih = np.asarray(W_ih, dtype=np.float32)
    W_hh = np.asarray(W_hh, dtype=np.float32)
    b_ih = np.asarray(b_ih, dtype=np.float32)
    b_hh = np.asarray(b_hh, dtype=np.float32)
    fc_w = np.asarray(fc_w, dtype=np.float32)
    fc_b = np.asarray(fc_b, dtype=np.float32)

    bc = h0 @ W_hh.T + b_ih + b_hh  # [64, 2048]
    wih_tp = np.ascontiguousarray(W_ih.T[0:512]).astype(ml_dtypes.bfloat16)
    wih_row_a = np.ascontiguousarray(W_ih.T[512:513]).astype(ml_dtypes.bfloat16)
    fc_pad = np.zeros((H, DPAD), dtype=ml_dtypes.bfloat16)
    fc_pad[:, 0:D] = fc_w.T.astype(ml_dtypes.bfloat16)
    fc_row_a = np.zeros((1, DPAD), dtype=ml_dtypes.bfloat16)
    fc_row_a[0, 0:D] = fc_b.astype(ml_dtypes.bfloat16)

    # x: bf16 cast + zero-pad to 640 cols (transpose windows of 128)
    x_pad = np.zeros((B * T, DX), dtype=ml_dtypes.bfloat16)
    x_pad[:, 0:D] = di.reshape(B * T, D).astype(ml_dtypes.bfloat16)

    in_maps = []
    for core in range(N_CORES):
        bs = core * B_LOC
        xc = x_pad[bs * T : (bs + B_LOC) * T]
        # bct[p, c*8+b] = bc[bs+b, c*128+p]
        bct = np.ascontiguousarray(
            bc[bs : bs + B_LOC]
            .reshape(B_LOC, 16, 128)
            .transpose(2, 1, 0)
            .reshape(128, -1)
        )
        c0c = np.ascontiguousarray(
            c0[bs : bs + B_LOC]
            .reshape(B_LOC, 4, 128)
            .transpose(2, 1, 0)
            .reshape(128, -1)
        )
        in_maps.append(
            {
                "x": xc,
                "wih_t": wih_tp,
                "wih_row": wih_row_a,
                "fc_rhs": fc_pad,
                "fc_row": fc_row_a,
                "bconst": bct,
                "c0t": c0c,
            }
        )
    return in_maps


def kernel(**inputs):
    in_maps = make_in_maps(**inputs)
    nc = get_nc()
    res = run_bass_kernel_spmd(nc, in_maps, core_ids=list(range(N_CORES)))
    out = np.concatenate([res.results[c]["out"] for c in range(N_CORES)], axis=0)
    return out.reshape(B, T, D)



# revision 43
# speedup vs baseline: 1.8200x; 1.8200x over previous
"""Trainium2 Bass kernel for nn_Decoder: fused single-step LSTM decoder.

Reference computation (per token t of batch b, state never advances):
    gates = x[b,t] @ W_ih.T + (h0[b] @ W_hh.T + b_ih + b_hh)     # [2048]
    i, f, g, o = sigmoid/sigmoid/tanh/sigmoid of gate quarters
    c = f * c0[b] + i * g
    h = o * tanh(c)
    out[b,t] = h @ fc_w.T + fc_b                                 # [513]

Sharding: data-parallel, batch 64 -> 8 batches per core on 8 NeuronCores.
Per-core layout strategy ("feature-major"), tuned against HW A/B timing
(the kernel is PE-stream-bound; each matmul stream costs its moving
cycles at 2.4 GHz plus ~83 ns of fixed issue overhead):
  - x features 0..511 are cast to bf16 on the host; each 512-token tile
    is brought in as 4 DMA xbar-transposes (DRAM [512 tok, 128 d] ->
    SBUF [128 d, 512]). Feature 512 is pre-transposed on the host into
    a tiny [tile, 1, 512] tensor (plain DMA) - no zero-padded 5th
    transpose window, which cuts input DMA bytes by 20%.
  - gates are computed transposed in bf16: gatesT[g-chunk, tok] with
    bf16 W_ihT stationary; K=513 = 4 full K=128 chunks + one K=1
    matmul. The per-batch bias const (h0 @ W_hh.T + b_ih + b_hh, fp32
    host precomputed) enters via the ScalarE activation bias AP.
    Gate psum is a 6-deep single-bank ring + single 2-bank fc psum
    ("ring6"): more PE run-ahead slack, measured -32us vs 4+2x2.
  - Activations write bf16; the LSTM cell runs feature-major in bf16 on
    DVE (2x mode) with c0 as a per-partition fp32 scalar; c lands in
    one [128, 4, 512] tile so tanh(c) is a single wide ScalarE call;
    h (bf16) lands directly in the [h, tok] layout fc needs as lhsT.
    The o-gate tiles get their own buffer ring (they outlive i/f/g,
    sharing a ring deadlocks the ScalarE/DVE streams).
  - fc runs in bf16 on h: out[tok, 513] token-major in two N-halves
    (258+258 into one 2-bank PSUM tile, cols 513..515 zero-pad). fc_b
    is added during the PSUM->SBUF move by DVE (tensor_add against a
    host-replicated [128, 516] fp32 bias tile) - no ones-row matmuls,
    no ScalarE copy. The fc matmuls of tile t are emitted AFTER the
    gate matmuls of tile t+1 ("lag-1 software pipelining") so the
    in-order PE stream never stalls on the act->cell->tanh->h chain;
    without this the kernel is ~200us slower.
  - Output is staged bf16 (halves the store bytes; +1e-3 rel err, well
    inside the 2e-2 budget) and written with one DMA per 512-token
    tile; the host upcasts to fp32.
  - All DMAs (including the xbar transposes) issue on the single SP
    queue. (Splitting copies onto the ACT HWDGE ring was measured
    correct - no xbar corruption - but not faster.)
  - An fp8e4 DoubleRow path (gfp8=True; error-compensated hi+lo split
    of x and W, accuracy 5.2e-3) is implemented but measured SLOWER
    than bf16 (~295ns per DR stream: the un-hidden DoubleRow
    LDWEIGHTS eats the 2x cycle win), so it is off by default.
"""

from contextlib import ExitStack

import ml_dtypes
import numpy as np

import concourse.bass as bass
import concourse.tile as tile
from concourse import bacc, mybir
from concourse.bass_utils import run_bass_kernel_spmd

FP32 = mybir.dt.float32
BF16 = mybir.dt.bfloat16
FP8 = mybir.dt.float8e4
U16 = mybir.dt.uint16
AFT = mybir.ActivationFunctionType
DR = mybir.MatmulPerfMode.DoubleRow
WSCALE = 16.0  # fp8 weight pre-scale (keeps W* out of the subnormal range)

N_CORES = 8
B, T, D = 64, 1024, 513
H = 512
G4 = 4 * H  # 2048
B_LOC = B // N_CORES  # 8 batches per core
TOK = B_LOC * T  # 8192 tokens per core
TT = 512  # tokens per tile (stays within one batch: T % TT == 0)
NT = TOK // TT  # 16 tiles
NM = TT // 128  # 4 token-subtiles of 128
DX = 640  # host-padded x width: 5 transpose windows of 128 (cols 513+ zero)
DPAD = 516  # fc output padded width (cols 513..515 zero garbage)
NSPLIT = [258, 258]  # fc N split halves (each fits one PSUM bank)


def build_nc(
    reps=1,
    ablate=(),
    lag=True,
    outq="sp",
    ring6=True,
    split_n=False,
    gfp8=False,
    slack=False,
):
    """ablate: subset of {"xpose", "out", "act", "dve"} for HW A/B timing.
    lag: emit tile t's fc after tile t+1's gates (hides the act->cell->h
    latency from the in-order PE stream).
    outq: engine whose HWDGE ring issues the output DMA ("sp" or "act").
    ring6: 6-deep gate psum ring + single-buffered fc psum (more PE slack)."""
    no_xpose = "xpose" in ablate
    no_out = "out" in ablate
    no_act = "act" in ablate
    no_dve = "dve" in ablate
    no_pe = "pe" in ablate  # DMAs only

    nc = bacc.Bacc("TRN2", target_bir_lowering=False, debug=False, num_devices=N_CORES)
    if gfp8:
        # x packed as u16 = (hi fp8 | lo fp8) per element; W split into
        # fp8 hi + residual, both pre-scaled by WSCALE (descaled in the
        # gate activation's scale) to stay clear of e4m3 subnormals.
        x = nc.dram_tensor("x", [TOK, H], U16, kind="ExternalInput").ap()
        x5 = nc.dram_tensor("x5", [NT, 1, 2, TT], FP8, kind="ExternalInput").ap()
        whi = nc.dram_tensor("whi", [128, 4, G4], FP8, kind="ExternalInput").ap()
        wlo = nc.dram_tensor("wlo", [128, 4, G4], FP8, kind="ExternalInput").ap()
        w5 = nc.dram_tensor("w5", [1, 2, G4], FP8, kind="ExternalInput").ap()
    else:
        x = nc.dram_tensor("x", [TOK, H], BF16, kind="ExternalInput").ap()
        x5 = nc.dram_tensor("x5", [NT, 1, TT], BF16, kind="ExternalInput").ap()
        wih_t = nc.dram_tensor("wih_t", [H, G4], BF16, kind="ExternalInput").ap()
        wih_row = nc.dram_tensor("wih_row", [1, G4], BF16, kind="ExternalInput").ap()
    fc_rhs = nc.dram_tensor("fc_rhs", [H, DPAD], BF16, kind="ExternalInput").ap()
    fcb_rep = nc.dram_tensor("fcb_rep", [128, DPAD], FP32, kind="ExternalInput").ap()
    bct = nc.dram_tensor("bconst", [128, 16 * B_LOC], FP32, kind="ExternalInput").ap()
    c0t = nc.dram_tensor("c0t", [128, 4 * B_LOC], FP32, kind="ExternalInput").ap()
    out = nc.dram_tensor("out", [TOK, D], BF16, kind="ExternalOutput").ap()

    with tile.TileContext(nc) as tc, ExitStack() as ctx:
        const = ctx.enter_context(tc.tile_pool(name="const", bufs=1))
        xtp = ctx.enter_context(tc.tile_pool(name="xt", bufs=20 if slack else 15))
        sigp = ctx.enter_context(tc.tile_pool(name="sig", bufs=12 if slack else 10))
        cthp = ctx.enter_context(tc.tile_pool(name="cth", bufs=4 if slack else 2))
        hp = ctx.enter_context(tc.tile_pool(name="h", bufs=12 if slack else 8))
        outp = ctx.enter_context(tc.tile_pool(name="osb", bufs=4 if slack else 3))
        # PSUM 8 banks: gates 4 single-bank + fc 2 x 2-bank
        # (ring6: gates 6 single-bank + fc 1 x 2-bank)
        gpp = ctx.enter_context(
            tc.tile_pool(name="gp", bufs=6 if ring6 else 4, space="PSUM")
        )
        fmp = ctx.enter_context(
            tc.tile_pool(name="fm", bufs=1 if ring6 else 2, space="PSUM")
        )

        def load_weights():
            if gfp8:
                whi_sb = const.tile([128, 4, G4], FP8, tag="whi")
                nc.sync.dma_start(whi_sb[:], whi)
                wlo_sb = const.tile([128, 4, G4], FP8, tag="wlo")
                nc.sync.dma_start(wlo_sb[:], wlo)
                w5_sb = const.tile([1, 2, G4], FP8, tag="w5")
                nc.sync.dma_start(w5_sb[:], w5)
                return (whi_sb, wlo_sb, w5_sb)
            wsb = []
            for k in range(4):
                w = const.tile([128, G4], BF16, tag=f"w{k}", name=f"w{k}")
                nc.sync.dma_start(w[:], wih_t[k * 128 : (k + 1) * 128, :])
                wsb.append(w)
            w4 = const.tile([1, G4], BF16, tag="w4")
            nc.sync.dma_start(w4[:], wih_row)
            wsb.append(w4)
            return wsb

        def emit_transposes(tt):
            """Bring in tile tt: 4 DMA xbar transposes (features 0..511,
            DRAM [512 tok, 128 d] -> SBUF [128 d, 512]) plus the tiny
            host-pretransposed feature-512 row."""
            ts = tt * TT
            if gfp8:
                # one [128, 4, 2*TT] fp8 tile; each u16 transpose drops a
                # window of (hi|lo)-interleaved fp8 pairs into it
                xp = xtp.tile([128, 4, 2 * TT], FP8, tag="xt")
                for k in range(4):
                    nc.sync.dma_start_transpose(
                        xp[:, k, :].bitcast(U16),
                        x[ts : ts + TT, k * 128 : (k + 1) * 128],
                    )
                t5 = xtp.tile([1, 2, TT], FP8, tag="x5")
                nc.sync.dma_start(t5[:], x5[tt])
                return (xp, t5)
            xt = []
            for k in range(4):
                t = xtp.tile([128, TT], BF16, tag="xt")
                nc.sync.dma_start_transpose(
                    t[:], x[ts : ts + TT, k * 128 : (k + 1) * 128]
                )
                xt.append(t)
            t5 = xtp.tile([1, TT], BF16, tag="x5")
            nc.sync.dma_start(t5[:], x5[tt, :, :])
            xt.append(t5)
            return xt

        # ---- startup, DMAs ordered by first use on the single queue ----
        if no_xpose:
            xst = []
            for k in range(5):
                t = const.tile([128, TT], BF16, tag=f"xs{k}", name=f"xs{k}")
                nc.vector.memset(t[:], 0.25)
                xst.append(t)
        else:
            xt_queue = [emit_transposes(0)]
        wsb = load_weights()

        bct_sb = const.tile([128, 16 * B_LOC], FP32, tag="bct")
        c0_sb = const.tile([128, 4 * B_LOC], FP32, tag="c0")
        nc.sync.dma_start(bct_sb[:], bct)
        nc.sync.dma_start(c0_sb[:], c0t)

        fcsb = []
        for k in range(4):
            w = const.tile([128, DPAD], BF16, tag=f"f{k}", name=f"f{k}")
            nc.sync.dma_start(w[:], fc_rhs[k * 128 : (k + 1) * 128, :])
            fcsb.append(w)
        fcb_sb = const.tile([128, DPAD], FP32, tag="fcb")
        nc.sync.dma_start(fcb_sb[:], fcb_rep)

        if not no_xpose:
            xt_queue.append(emit_transposes(1))

        if no_act:
            hst = []
            for k in range(4):
                t = const.tile([128, TT], BF16, tag=f"hs{k}", name=f"hs{k}")
                nc.vector.memset(t[:], 0.25)
                hst.append(t)

        def gates_matmuls_fp8(sl, xt, c):
            """7 DoubleRow fp8 streams for gate-chunk c: x_hi@W_hi (2),
            x_lo@W_hi (2), x_hi@W_lo (2), feature-512 (hi,lo)@W_hi (1).
            All at 0.5 cyc/row; psum holds WSCALE * gates."""
            whi_sb, wlo_sb, w5_sb = wsb
            xp, t5 = xt
            csl = slice(c * 128, (c + 1) * 128)
            planes = xp.rearrange("p w (t two) -> p w two t", two=2)
            first = True
            for wt, plane in ((whi_sb, 0), (whi_sb, 1), (wlo_sb, 0)):
                for pr in range(2):
                    nc.tensor.matmul(
                        sl,
                        wt[:, 2 * pr : 2 * pr + 2, csl],
                        planes[:, 2 * pr : 2 * pr + 2, plane, :],
                        start=first,
                        stop=False,
                        perf_mode=DR,
                    )
                    first = False
            nc.tensor.matmul(
                sl,
                w5_sb[:, :, csl],
                t5[:],
                start=False,
                stop=True,
                perf_mode=DR,
            )

        def gates_matmuls(sl, xt, c):
            """5 K-chunk matmuls for gate-chunk c into psum slice sl.
            (split_n: same cycles as 2x N=256 streams - overhead probe)"""
            if gfp8:
                return gates_matmuls_fp8(sl, xt, c)
            if split_n:
                for half in range(2):
                    hs = sl[:, half * 256 : (half + 1) * 256]
                    for kd in range(4):
                        nc.tensor.matmul(
                            hs,
                            wsb[kd][:, c * 128 : (c + 1) * 128],
                            xt[kd][:, half * 256 : (half + 1) * 256],
                            start=(kd == 0),
                            stop=False,
                        )
                    nc.tensor.matmul(
                        hs,
                        wsb[4][:, c * 128 : (c + 1) * 128],
                        xt[4][0:1, half * 256 : (half + 1) * 256],
                        start=False,
                        stop=True,
                    )
                return
            for kd in range(4):
                nc.tensor.matmul(
                    sl,
                    wsb[kd][:, c * 128 : (c + 1) * 128],
                    xt[kd][:],
                    start=(kd == 0),
                    stop=False,
                )
            nc.tensor.matmul(
                sl,
                wsb[4][:, c * 128 : (c + 1) * 128],
                xt[4][0:1, :],
                start=False,
                stop=True,
            )

        gate_funcs = [AFT.Sigmoid, AFT.Sigmoid, AFT.Tanh, AFT.Sigmoid]

        def emit_fc(hn, ts):
            # fc: out[tok, 513] per 128-token subtile, N split 258 + 258
            osb = outp.tile([128, NM, DPAD], BF16, tag="osb")
            for m in range(NM):
                msl = slice(m * 128, (m + 1) * 128)
                pf = fmp.tile([128, 1024], FP32, tag="fm")
                for kd in range(4):
                    nc.tensor.matmul(
                        pf[:, 0 : NSPLIT[0]],
                        hn[kd][:, msl],
                        fcsb[kd][:, 0 : NSPLIT[0]],
                        start=(kd == 0),
                        stop=(kd == 3),
                    )
                for kd in range(4):
                    nc.tensor.matmul(
                        pf[:, 512 : 512 + NSPLIT[1]],
                        hn[kd][:, msl],
                        fcsb[kd][:, NSPLIT[0] : DPAD],
                        start=(kd == 0),
                        stop=(kd == 3),
                    )
                # psum -> sbuf with fc_b folded in (DVE)
                nc.vector.tensor_add(
                    osb[:, m, 0:DPAD].rearrange("p (a b) -> p a b", a=2),
                    pf[:].rearrange("p (a b) -> p a b", a=2)[:, :, 0 : NSPLIT[0]],
                    fcb_sb[:].rearrange("p (a b) -> p a b", a=2),
                )
            if not no_out:
                dma_eng = nc.scalar if outq == "act" else nc.sync
                dma_eng.dma_start(
                    out[ts : ts + TT, :].rearrange("(m p) d -> p m d", p=128),
                    osb[:, :, 0:D],
                )

        # ---- main loop over 16 token tiles of 512 ----
        pend_fc = None
        rep_ctx = tc.For_i(0, reps, 1) if reps > 1 else None
        if rep_ctx is not None:
            rep_ctx.__enter__()
        for tt in range(NT):
            b = tt // (T // TT)
            ts = tt * TT
            if no_xpose:
                xt = xst
            else:
                xt = xt_queue.pop(0)
                # prefetch + transpose two tiles ahead while this tile computes
                if reps > 1:
                    xt_queue.append(emit_transposes((tt + 2) % NT))
                elif tt + 2 < NT:
                    xt_queue.append(emit_transposes(tt + 2))

            if no_pe:
                osb = outp.tile([128, NM, DPAD], BF16, tag="osb")
                nc.vector.memset(osb[:, :, 0:4], 0.0)
                if not no_out:
                    nc.sync.dma_start(
                        out[ts : ts + TT, :].rearrange("(m p) d -> p m d", p=128),
                        osb[:, :, 0:D],
                    )
                continue

            # gates + LSTM cell, per h-chunk k; gate-chunk index: gi*4+k
            if no_act:
                hn = hst
                # keep the same matmul work, but discard into rotating psum
                for k in range(4):
                    for gi in range(4):
                        ps = gpp.tile([128, TT], FP32, tag="gp")
                        gates_matmuls(ps[:], xt, gi * 4 + k)
            else:
                call = cthp.tile([128, 4, TT], BF16, tag="call")
                osg = []
                for k in range(4):
                    gs = []
                    for gi in range(4):
                        c = gi * 4 + k
                        ps = gpp.tile([128, TT], FP32, tag="gp")
                        gates_matmuls(ps[:], xt, c)
                        # o lives until the end-of-tile h-muls: own ring so
                        # i/f/g reuse can never wait on (and deadlock with) it
                        if gi == 3:
                            s = sigp.tile([128, TT], BF16, tag="o", bufs=8)
                        else:
                            s = sigp.tile([128, TT], BF16, tag="sig")
                        nc.scalar.activation(
                            s[:],
                            ps[:],
                            gate_funcs[gi],
                            bias=bct_sb[:, c * B_LOC + b : c * B_LOC + b + 1],
                            scale=(1.0 / WSCALE) if gfp8 else 1.0,
                        )
                        gs.append(s)
                    i_s, f_s, g_s, o_s = gs
                    osg.append(o_s)
                    if no_dve:
                        continue
                    t1 = cthp.tile([128, TT], BF16, tag="t1")
                    nc.vector.tensor_mul(t1[:], i_s[:], g_s[:])
                    t2 = cthp.tile([128, TT], BF16, tag="t2")
                    nc.vector.tensor_scalar_mul(
                        t2[:], f_s[:], c0_sb[:, k * B_LOC + b : k * B_LOC + b + 1]
                    )
                    nc.vector.tensor_add(call[:, k, :], t1[:], t2[:])
                if no_dve:
                    hn = osg
                else:
                    thall = cthp.tile([128, 4, TT], BF16, tag="thall")
                    nc.scalar.activation(thall[:], call[:], AFT.Tanh)
                    hn = []
                    for k in range(4):
                        h = hp.tile([128, TT], BF16, tag="h")
                        nc.vector.tensor_mul(h[:], osg[k][:], thall[:, k, :])
                        hn.append(h)

            # fc of the PREVIOUS tile is emitted after this tile's gates so
            # the in-order PE stream never waits on the act->cell->h chain.
            if lag:
                if pend_fc is not None:
                    emit_fc(*pend_fc)
                pend_fc = (hn, ts)
            else:
                emit_fc(hn, ts)
        if pend_fc is not None:
            emit_fc(*pend_fc)
        if rep_ctx is not None:
            rep_ctx.__exit__(None, None, None)

    nc.compile()
    return nc


_NC_CACHE = []


def get_nc():
    if not _NC_CACHE:
        _NC_CACHE.append(build_nc(gfp8=GFP8))
    return _NC_CACHE[0]


GFP8 = False  # module default for kernel(); build_nc/make_in_maps must agree


def make_in_maps(
    decoder_inputs, h0, c0, W_ih, W_hh, b_ih, b_hh, fc_w, fc_b, gfp8=None
):
    if gfp8 is None:
        gfp8 = GFP8
    di = np.asarray(decoder_inputs, dtype=np.float32)
    h0 = np.asarray(h0, dtype=np.float32)[0]  # [64, 512]
    c0 = np.asarray(c0, dtype=np.float32)[0]
    W_ih = np.asarray(W_ih, dtype=np.float32)
    W_hh = np.asarray(W_hh, dtype=np.float32)
    b_ih = np.asarray(b_ih, dtype=np.float32)
    b_hh = np.asarray(b_hh, dtype=np.float32)
    fc_w = np.asarray(fc_w, dtype=np.float32)
    fc_b = np.asarray(fc_b, dtype=np.float32)

    bc = h0 @ W_hh.T + b_ih + b_hh  # [64, 2048]
    wih_tp = np.ascontiguousarray(W_ih.T[0:512]).astype(ml_dtypes.bfloat16)
    wih_row_a = np.ascontiguousarray(W_ih.T[512:513]).astype(ml_dtypes.bfloat16)
    fc_pad = np.zeros((H, DPAD), dtype=ml_dtypes.bfloat16)
    fc_pad[:, 0:D] = fc_w.T.astype(ml_dtypes.bfloat16)
    fcb_rep_a = np.zeros((128, DPAD), dtype=np.float32)
    fcb_rep_a[:, 0:D] = fc_b[None, :]

    e4 = ml_dtypes.float8_e4m3
    if gfp8:
        # x packed per element as u16 = (hi fp8 | lo fp8); feature 512
        # pre-transposed per tile as fp8 (hi plane, lo plane)
        x32 = di.reshape(B * T, D)
        hi = x32[:, 0:H].astype(e4)
        lo = (x32[:, 0:H] - hi.astype(np.float32)).astype(e4)
        x_main = (
            hi.view(np.uint8).astype(np.uint16)
            | (lo.view(np.uint8).astype(np.uint16) << 8)
        )
        h5 = x32[:, H].astype(e4)
        l5 = (x32[:, H] - h5.astype(np.float32)).astype(e4)
        x_last = np.stack(
            [h5.reshape(-1, TT), l5.reshape(-1, TT)], axis=1
        ).reshape(B * T // TT, 1, 2, TT)
        # weights pre-scaled by WSCALE, split hi + residual
        ws = (W_ih * WSCALE)[:, 0:H].T.reshape(4, 128, G4).transpose(1, 0, 2)
        whi_a = np.ascontiguousarray(ws).astype(e4)
        wlo_a = (ws - whi_a.astype(np.float32)).astype(e4)
        w5_a = np.broadcast_to(
            (W_ih[:, H] * WSCALE).astype(e4), (1, 2, G4)
        ).copy()
    else:
        # x: bf16 cast; features 0..511 go through the DMA transposes, the
        # 513th feature is pre-transposed on the host ([tile, 1, tok]).
        x_bf = di.reshape(B * T, D).astype(ml_dtypes.bfloat16)
        x_main = np.ascontiguousarray(x_bf[:, 0:H])
        x_last = np.ascontiguousarray(x_bf[:, H]).reshape(B * T // TT, 1, TT)

    in_maps = []
    for core in range(N_CORES):
        bs = core * B_LOC
        xc = x_main[bs * T : (bs + B_LOC) * T]
        x5c = x_last[bs * T // TT : (bs + B_LOC) * T // TT]
        # bct[p, c*8+b] = bc[bs+b, c*128+p]
        bct = np.ascontiguousarray(
            bc[bs : bs + B_LOC]
            .reshape(B_LOC, 16, 128)
            .transpose(2, 1, 0)
            .reshape(128, -1)
        )
        c0c = np.ascontiguousarray(
            c0[bs : bs + B_LOC]
            .reshape(B_LOC, 4, 128)
            .transpose(2, 1, 0)
            .reshape(128, -1)
        )
        m = {
            "x": xc,
            "x5": x5c,
            "fc_rhs": fc_pad,
            "fcb_rep": fcb_rep_a,
            "bconst": bct,
            "c0t": c0c,
        }
        if gfp8:
            m.update({"whi": whi_a, "wlo": wlo_a, "w5": w5_a})
        else:
            m.update({"wih_t": wih_tp, "wih_row": wih_row_a})
        in_maps.append(m)
    return in_maps


def kernel(**inputs):
    in_maps = make_in_maps(**inputs)
    nc = get_nc()
    res = run_bass_kernel_spmd(nc, in_maps, core_ids=list(range(N_CORES)))
    out = np.concatenate(
        [np.asarray(res.results[c]["out"]).astype(np.float32) for c in range(N_CORES)],
        axis=0,
    )
    return out.reshape(B, T, D)
